# revision 8
# baseline (speedup 1.0000x reference)
"""DCNv2 (modulated deformable convolution) on 8 Trainium2 NeuronCores.

kernel(**inputs) takes the full unsharded inputs
    x      (8, 128, 64, 64) f32
    w_om   (27, 128, 3, 3)  f32
    b_om   (27,)            f32
    weight (128, 128, 3, 3) f32
    bias   (128,)           f32
and returns the full output (8, 128, 64, 64) f32.

Sharding: pure data-parallel over batch - one image per NeuronCore, small
weights replicated; no collectives.

v9 per-core program (bf16 datapath):
  1. x/weights are cast to bf16 during the load DMA (SWDGE); x is staged
     twice into a DRAM image xt2[GROWS, 256] where row r=(y,x) holds
     [C(y,x), C(y+1,x)] - the 4 bilinear corners of any sample are 4*128
     CONTIGUOUS bf16 values (one 1KB gather descriptor per (pixel, tap)).
     Staging is fully DMA/PE-light: 32 PE transposes land row-pair tiles
     in PSUM, the DVE duplicates them into (B|A) pairs, and 16 big strided
     DMAs write both copies; pad rows are zero-filled separately (disjoint
     regions, no WAW with the staged interior).
  2. offset conv (27ch 3x3) on the PE in bf16, split into two halves of
     the image so the whole index pipeline (sampling positions -> gather
     row indices -> 16-partition dma_gather wrap) completes for half 0
     while half 1 is still in the offset conv; gather descriptor
     generation (the serial Q7 bottleneck) starts ~50us earlier than a
     monolithic setup would allow.
  3. per (half, tap) one dma_gather (SWDGE queues round-robined 0-3)
     fetches [A0 B0 A1 B1] corner blocks in (pixel-partition, channel)
     layout; corners are combined at whole-gather granularity: ACT applies
     c00 per pixel-tile, DVE does 3 broadcast-coefficient multiplies (bf16
     coefficients - pure-bf16 ops keep the DVE 2x datapath) + 3 bf16 adds;
     the result is PE-transposed back to (channel, pixel) and accumulated
     over the 9 taps into PSUM with the 128x128x3x3 weight; bias is added
     on the PSUM->SBUF copy.
"""

import os
import sys

import numpy as np

sys.path.insert(0, "/opt/trn_rl_repo")

from contextlib import ExitStack

import concourse.bacc as bacc
import concourse.mybir as mybir
import concourse.tile as tile
from concourse._compat import get_trn_type
from concourse.alu_op_type import AluOpType as Alu
from concourse.bass import AP
from concourse.bass_utils import run_bass_kernel_spmd
from concourse import library_config

F32 = mybir.dt.float32
BF16 = mybir.dt.bfloat16
I32 = mybir.dt.int32
I16 = mybir.dt.int16

B = 8
C = 128
H = W = 64
HW = H * W
K2 = 9
PADG = 4
GW = H + 2 * PADG      # 72
GROWS = GW * GW        # 5184
NS = 32
NHALF = 2
SPH = NS // NHALF      # 16 s-tiles per half
PPH = HW // NHALF      # 2048 pixels per half
IDENT = mybir.ActivationFunctionType.Identity

LAST_EXEC_TIME_NS = None
LAST_RESULT = None
SINGLE_PACKET = bool(int(os.environ.get("DCN_SP", "0")))


def _emit(tc):
    nc = tc.nc
    x_d = nc.dram_tensor("x", [C, HW], F32, kind="ExternalInput").ap()
    w_om_d = nc.dram_tensor("w_om", [27, 1152], F32, kind="ExternalInput").ap()
    b_om_d = nc.dram_tensor("b_om", [27, 1], F32, kind="ExternalInput").ap()
    weight_d = nc.dram_tensor("weight", [C, 1152], F32, kind="ExternalInput").ap()
    bias_d = nc.dram_tensor("bias", [C, 1], F32, kind="ExternalInput").ap()
    out_d = nc.dram_tensor("out", [C, HW], F32, kind="ExternalOutput").ap()
    xt2_d = nc.dram_tensor("xt2_pad", [GROWS, 256], BF16, kind="Internal").ap()
    consts_d = nc.dram_tensor("consts", [128, 707], F32, kind="ExternalInput").ap()

    ctx = ExitStack()
    with ctx:
        cpool = ctx.enter_context(tc.tile_pool(name="const", bufs=1))
        spool = ctx.enter_context(tc.tile_pool(name="setup", bufs=1))
        stgpool = ctx.enter_context(tc.tile_pool(name="stg", bufs=2))
        dpool = ctx.enter_context(tc.tile_pool(name="data", bufs=1))
        gpool = ctx.enter_context(tc.tile_pool(name="gath", bufs=4))
        vpool = ctx.enter_context(tc.tile_pool(name="val", bufs=2))
        ppool = ctx.enter_context(tc.tile_pool(name="psum", bufs=1, space="PSUM"))
        tpool = ctx.enter_context(tc.tile_pool(name="trps", bufs=2, space="PSUM"))
        opool = ctx.enter_context(tc.tile_pool(name="omps", bufs=2, space="PSUM"))

        # ---------- loads ----------
        cons = cpool.tile([128, 707], F32)
        nc.sync.dma_start(cons[:], consts_d[:, :])
        ident = cons[:, 0:128]
        hob = cons[:, 129:130]
        wo_r = cons[:, 130:131]
        ykc = cons[:, 131:419]
        xkc = cons[:, 419:707]

        x16 = spool.tile([128, HW], BF16)
        nc.gpsimd.dma_start(x16[:], x_d[:, :])
        w_om16 = spool.tile([27, 1152], BF16)
        nc.gpsimd.dma_start(w_om16[:], w_om_d[:, :])
        w16 = spool.tile([128, 1152], BF16)
        nc.gpsimd.dma_start(w16[:], weight_d[:, :])
        nc.gpsimd.load_library(library_config.mlp)

        b_om_sb = spool.tile([27, 1], F32)
        nc.sync.dma_start(b_om_sb[:], b_om_d[:, :])
        bias_sb = spool.tile([128, 1], F32)
        nc.sync.dma_start(bias_sb[:], bias_d[:, :])

        identb = spool.tile([128, 128], BF16)
        nc.vector.tensor_copy(identb[:], ident)

        # ---------- zero-fill xt2 pad regions (disjoint from staged rows) --
        zt = spool.tile([128, 576], BF16)
        nc.vector.memset(zt[:], 0.0)
        # top rows 0..3 (incl. row 3 cols 0:128 = C(-1); cols 128:256 of row
        # 3 are overwritten by the B-copy of chunk 0 afterwards)
        nc.scalar.dma_start(AP(xt2_d.tensor, 0, [[576, 128], [1, 576]]), zt[:])
        # bottom rows 68..71
        nc.scalar.dma_start(
            AP(xt2_d.tensor, 68 * GW * 256, [[576, 128], [1, 576]]), zt[:])
        # left pad cols x'=0..3 of interior rows
        nc.scalar.dma_start(
            AP(xt2_d.tensor, 4 * GW * 256, [[GW * 256, 64], [1, 1024]]),
            zt[:, 0:512])
        # right pad cols x'=68..71 of interior rows
        nc.scalar.dma_start(
            AP(xt2_d.tensor, (4 * GW + 68) * 256, [[GW * 256, 64], [1, 1024]]),
            zt[:, 0:512])
        # row 67 cols 128:256 = C(64) = 0 (not covered by B-copies)
        nc.scalar.dma_start(
            AP(xt2_d.tensor, (67 * GW + 4) * 256 + 128, [[256, 64], [1, 128]]),
            zt[0:64, 0:128])

        # ---------- stage xt2 interior ----------
        # xt2 row r = (y+PADG, x+PADG) holds [C(y,x), C(y+1,x)].  Chunk
        # s covers image rows (2s, 2s+1): PE-transpose to [pixel, chan],
        # copy to SBUF, then per (4-chunk group, y2-parity) two strided
        # DMAs write the same slab twice:
        #   A: row 2s+4+y2 cols   0:128  = C(2s+y2)
        #   B: row 2s+3+y2 cols 128:256  = C(2s+y2)
        for g in range(8):
            trp4 = tpool.tile([128, 512], BF16, tag="tr", name="trp")
            for s4 in range(4):
                s = 4 * g + s4
                nc.tensor.transpose(
                    trp4[:, 128 * s4:128 * s4 + 128],
                    x16[:, 128 * s:128 * s + 128], identb[:])
            stg4 = stgpool.tile([128, 512], BF16, tag="stg", name="stg")
            nc.vector.tensor_copy(stg4[:], trp4[:])
            stg4v = stg4[:].rearrange("p (a b) -> p a b", a=4)
            for y2 in range(2):
                eng = nc.sync if y2 == 0 else nc.scalar
                src = stg4v[64 * y2:64 * y2 + 64, :, :]
                eng.dma_start(
                    AP(xt2_d.tensor, ((8 * g + 4 + y2) * GW + 4) * 256,
                       [[256, 64], [2 * GW * 256, 4], [1, 128]]),
                    src,
                )
                eng.dma_start(
                    AP(xt2_d.tensor, ((8 * g + 3 + y2) * GW + 4) * 256 + 128,
                       [[256, 64], [2 * GW * 256, 4], [1, 128]]),
                    src,
                )

        # ---------- x_pad (bf16) for the offset conv ----------
        XP = 66
        x_pad = spool.tile([128, XP * XP], BF16)
        xpv = x_pad[:].rearrange("p (a b) -> p a b", a=XP)
        nc.vector.memset(xpv[:, 0:1, :], 0.0)
        nc.vector.memset(xpv[:, 65:66, :], 0.0)
        nc.vector.memset(xpv[:, 1:65, 0:1], 0.0)
        nc.vector.memset(xpv[:, 1:65, 65:66], 0.0)
        nc.vector.tensor_copy(
            xpv[:, 1:65, 1:65],
            x16[:].rearrange("p (a b) -> p a b", a=64),
        )

        # ---------- weight transposes (offset conv) ----------
        womT = spool.tile([128, K2, 27], BF16)
        for k in range(K2):
            trp = tpool.tile([128, 512], BF16, tag="tr", name="trp")
            nc.tensor.transpose(
                trp[:, 0:27],
                w_om16[:].rearrange("p (c k) -> p c k", k=K2)[:, :, k],
                identb[0:27, 0:27],
            )
            nc.scalar.copy(womT[:, k, :], trp[:, 0:27])

        om_sb = spool.tile([27, HW], BF16)
        omT = spool.tile([128, NS, 27], F32)
        idxAw = spool.tile([128, K2 * 256], I16)
        omT_t = omT[:].tensor
        omT_off = omT[:].offset

        _cnt = [0]

        def f(shape=(128, SPH, K2), dt=F32, tag=None):
            _cnt[0] += 1
            nm = f"cf{_cnt[0]}"
            return dpool.tile(list(shape), dt, tag=tag or nm, name=nm)

        def om_conv_half(h):
            # offset conv chunks + per-s4 omT transposes for one half
            for ch in range(4 * h, 4 * h + 4):
                omp = opool.tile([128, 512], F32, tag="om", name="omp")
                for k in range(K2):
                    dy_, dx_ = k // 3, k % 3
                    r0 = ch * 8 + dy_
                    nc.tensor.matmul(
                        omp[0:27, :], womT[:, k, :],
                        xpv[:, r0:r0 + 8, dx_:dx_ + 64],
                        start=(k == 0), stop=(k == K2 - 1),
                    )
                nc.scalar.activation(
                    om_sb[:, ch * 512:(ch + 1) * 512], omp[0:27, :],
                    IDENT, bias=b_om_sb[:], scale=1.0,
                )
                # omT (128 pix, 27) for the 4 s-tiles of this chunk
                trp = tpool.tile([128, 512], BF16, tag="tr", name="trp")
                for j in range(4):
                    nc.tensor.transpose(
                        trp[:, j * 128:j * 128 + 27],
                        om_sb[:, (4 * ch + j) * 128:(4 * ch + j + 1) * 128],
                        identb[0:27, 0:27],
                    )
                nc.scalar.copy(
                    omT[:, 4 * ch:4 * ch + 4, :],
                    trp[:].rearrange("p (a b) -> p a b", b=128)[:, :, 0:27],
                )

        def index_half(h):
            # sampling positions -> gather row indices for s-tiles
            # h*16..h*16+15; returns (py, px, y0f, x0f, mlg)
            so = h * SPH
            dyT = AP(omT_t, omT_off + so * 27,
                     [[NS * 27, 128], [27, SPH], [2, K2]])
            dxT = AP(omT_t, omT_off + so * 27 + 1,
                     [[NS * 27, 128], [27, SPH], [2, K2]])
            ykv = ykc.rearrange("p (s a) -> p s a", a=K2)[:, so:so + SPH, :]
            xkv = xkc.rearrange("p (s a) -> p s a", a=K2)[:, so:so + SPH, :]
            py = f(tag=f"py{h}")
            nc.vector.scalar_tensor_tensor(py[:], dyT, hob, ykv, Alu.add,
                                           Alu.add)
            px = f(tag=f"px{h}")
            nc.vector.scalar_tensor_tensor(px[:], dxT, wo_r, xkv, Alu.add,
                                           Alu.add)

            def floorit(v, nm):
                vi = f(dt=I32, tag=f"fl_i{h}")
                nc.vector.tensor_copy(vi[:], v[:])
                v0 = f(tag=f"fl_f{h}")
                nc.vector.tensor_copy(v0[:], vi[:])
                gt = f(tag=f"fl_gt{h}")
                nc.vector.tensor_tensor(gt[:], v0[:], v[:], Alu.is_gt)
                v0f = f(tag=nm)
                nc.vector.tensor_tensor(v0f[:], v0[:], gt[:], Alu.subtract)
                return v0f

            y0f = floorit(py, f"y0f{h}")
            x0f = floorit(px, f"x0f{h}")
            nc.vector.tensor_scalar(y0f[:], y0f[:], -float(PADG), float(H + 2),
                                    Alu.max, Alu.min)
            nc.vector.tensor_scalar(x0f[:], x0f[:], -float(PADG), float(W + 2),
                                    Alu.max, Alu.min)

            # row index r = (y0+PADG)*GW + (x0+PADG), k-major: gKM[p][k][s]
            gAf = f(tag=f"gA{h}")
            nc.vector.tensor_scalar(gAf[:], y0f[:], float(GW),
                                    float(PADG * GW + PADG),
                                    Alu.mult, Alu.add)
            gKM = dpool.tile([128, K2, SPH], F32, tag=f"gKM{h}",
                             name=f"gKM{h}")
            gKM_w = AP(gKM[:].tensor, gKM[:].offset,
                       [[K2 * SPH, 128], [1, SPH], [SPH, K2]])
            nc.vector.tensor_tensor(gKM_w, gAf[:], x0f[:], Alu.add)
            return py, px, y0f, x0f, gKM

        def wrap_half(h, gKM):
            # idx wrap via PE transposes: idxAw[16u+pp][k*256 + h*128 +
            # s*8 + u] = gKM[16u+pp][k][s]
            t1s = []
            gv = gKM[:].rearrange("p a b -> p (a b)")
            for g in range(3):  # pass 1: [128, 48] -> [48, 128]
                trp = opool.tile([128, 512], F32, tag="om", name="omp")
                nc.tensor.transpose(
                    trp[0:48, 0:128], gv[:, 48 * g:48 * (g + 1)], ident[:])
                t1 = spool.tile([48, 128], F32, tag=f"t1_{h}_{g}")
                nc.scalar.copy(t1[:], trp[0:48, 0:128])
                t1s.append(t1)
            for g in range(3):
                for u4 in range(2):  # pass 2: 4x [48, 16] -> [16, 48]
                    trp = opool.tile([128, 512], F32, tag="om", name="omp")
                    for j in range(4):
                        u = 4 * u4 + j
                        nc.tensor.transpose(
                            trp[0:16, j * 128:j * 128 + 48],
                            t1s[g][:, 16 * u:16 * u + 16],
                            ident[0:48, 0:48],
                        )
                    t2 = vpool.tile([16, 512], F32, tag="t2", name="t2")
                    nc.scalar.copy(t2[:], trp[0:16, :])
                    # scatter (j, k', s) -> col (3g+k')*256 + h*128 + s*8 + u
                    dst = AP(
                        idxAw[:].tensor,
                        idxAw[:].offset + (3 * g) * 256 + 128 * h + 4 * u4,
                        [[K2 * 256, 16], [1, 4], [256, 3], [8, SPH]],
                    )
                    src = AP(
                        t2[:].tensor, t2[:].offset,
                        [[512, 16], [128, 4], [16, 3], [1, 16]],
                    )
                    nc.vector.tensor_copy(dst, src)
            # replicate this half's index columns to all 8 16-row groups
            iv = idxAw[:].rearrange("p (k g c) -> p k g c", k=K2, g=2)
            for u2 in range(1, 8):
                nc.scalar.dma_start(
                    iv[16 * u2:16 * u2 + 16, :, h, :], iv[0:16, :, h, :])

        def coef_half(h, py, px, y0f, x0f):
            # softmax mask + bilinear coefficients (c00 f32 for the ACT
            # scale; the rest bf16 so the DVE combine stays pure-bf16)
            mlg = omT[:, h * SPH:(h + 1) * SPH, 18:27]
            e = f(tag=f"e{h}")
            nc.scalar.activation(e[:], mlg, mybir.ActivationFunctionType.Exp)
            ssum = f((128, SPH, 1), tag=f"ss{h}")
            nc.vector.tensor_reduce(ssum[:], e[:], mybir.AxisListType.X,
                                    Alu.add)
            rs = f((128, SPH, 1), tag=f"rs{h}")
            nc.vector.reciprocal(rs[:], ssum[:])
            mask = f(tag=f"mask{h}")
            nc.vector.tensor_tensor(mask[:], e[:],
                                    rs[:].to_broadcast([128, SPH, K2]),
                                    Alu.mult)

            wy1 = f(tag=f"wy1{h}")
            nc.vector.tensor_tensor(wy1[:], py[:], y0f[:], Alu.subtract)
            wy0 = f(tag=f"wy0{h}")
            nc.vector.tensor_scalar(wy0[:], wy1[:], -1.0, 1.0, Alu.mult,
                                    Alu.add)
            wx1 = f(tag=f"wx1{h}")
            nc.vector.tensor_tensor(wx1[:], px[:], x0f[:], Alu.subtract)
            wx0 = f(tag=f"wx0{h}")
            nc.vector.tensor_scalar(wx0[:], wx1[:], -1.0, 1.0, Alu.mult,
                                    Alu.add)

            mwy0 = f(tag=f"mwy0{h}")
            nc.vector.tensor_tensor(mwy0[:], mask[:], wy0[:], Alu.mult)
            mwy1 = f(tag=f"mwy1{h}")
            nc.vector.tensor_tensor(mwy1[:], mask[:], wy1[:], Alu.mult)
            c00 = f(dt=BF16, tag=f"c00{h}")
            nc.vector.tensor_tensor(c00[:], mwy0[:], wx0[:], Alu.mult)
            c01 = f(dt=BF16, tag=f"c01{h}")
            nc.vector.tensor_tensor(c01[:], mwy0[:], wx1[:], Alu.mult)
            c10 = f(dt=BF16, tag=f"c10{h}")
            nc.vector.tensor_tensor(c10[:], mwy1[:], wx0[:], Alu.mult)
            c11 = f(dt=BF16, tag=f"c11{h}")
            nc.vector.tensor_tensor(c11[:], mwy1[:], wx1[:], Alu.mult)
            return c00, c01, c10, c11

        # ---------- setup, half 0 first so gathers start early ----------
        om_conv_half(0)
        py0, px0, y0f0, x0f0, gKM0 = index_half(0)
        wrap_half(0, gKM0)
        cs0 = coef_half(0, py0, px0, y0f0, x0f0)

        om_conv_half(1)

        wT = spool.tile([128, K2, 128], BF16)
        for k in range(K2):
            trp = tpool.tile([128, 512], BF16, tag="tr", name="trp")
            nc.tensor.transpose(
                trp[:, 0:128],
                w16[:].rearrange("p (c k) -> p c k", k=K2)[:, :, k], identb[:],
            )
            nc.scalar.copy(wT[:, k, :], trp[:, 0:128])

        py1, px1, y0f1, x0f1, gKM1 = index_half(1)
        wrap_half(1, gKM1)
        cs1 = coef_half(1, py1, px1, y0f1, x0f1)
        coefs = [cs0, cs1]

        # Gate the gather storm on the full DVE index/coef pipeline: small
        # int-cast/clamp DVE ops that run concurrently with DMAGatherAnt
        # descriptor generation block until the generation finishes (SBUF
        # ring arbitration), stalling the DVE and starving the combine.
        # Rewriting idxAw with +0 (derived from the last coefficient tile)
        # gives every dma_gather a data dependency on the completed setup.
        zi = dpool.tile([128, 1], I16, tag="zi", name="zi")
        nc.vector.tensor_tensor(zi[:], cs1[3][:, 0, 0:1], cs1[3][:, 0, 0:1],
                                Alu.subtract)
        nc.vector.tensor_tensor(idxAw[:], idxAw[:],
                                zi[:].to_broadcast([128, K2 * 256]), Alu.add)

        # ---------- main loop ----------
        out_sb = spool.tile([128, HW], F32)
        xt2_src = AP(xt2_d.tensor, 0, [[256, GROWS - 1], [1, 512]])
        for h in range(NHALF):
            c00, c01, c10, c11 = coefs[h]
            outp = ppool.tile([128, PPH], F32, tag="out", name="outp")
            for k in range(K2):
                # split the last tap (tail) and the very first gather
                # (startup ramp) into two half-gathers on separate queues
                split = (k == K2 - 1) or (h == 0 and k == 0)
                parts = ((0, 8), (8, SPH)) if split else ((0, SPH),)
                for (t0, t1) in parts:
                    nt = t1 - t0
                    gb = gpool.tile([128, nt, 512], BF16, tag="gb", name="gb")
                    nc.gpsimd.dma_gather(
                        gb[:], xt2_src,
                        idxAw[:, k * 256 + 128 * h + 8 * t0:
                              k * 256 + 128 * h + 8 * t0 + 8 * nt],
                        128 * nt, 128 * nt, 512, elem_step=256,
                        single_packet=SINGLE_PACKET,
                        queue_num=(h * K2 + k + t0 // 8) % 4,
                    )
                    # corners: [0:128]=A0(c00) [128:256]=B0(c10)
                    #          [256:384]=A1(c01) [384:512]=B1(c11)
                    mb = vpool.tile([128, nt, 128], BF16, tag="mb", name="mb")
                    nc.vector.tensor_tensor(
                        mb[:], gb[:, :, 0:128],
                        c00[:, t0:t0 + nt, k:k + 1].to_broadcast(
                            [128, nt, 128]),
                        Alu.mult)
                    u1 = vpool.tile([128, nt, 128], BF16, tag="u1", name="u1")
                    nc.vector.tensor_tensor(
                        u1[:], gb[:, :, 256:384],
                        c01[:, t0:t0 + nt, k:k + 1].to_broadcast(
                            [128, nt, 128]),
                        Alu.mult)
                    u2 = vpool.tile([128, nt, 128], BF16, tag="u2", name="u2")
                    nc.vector.tensor_tensor(
                        u2[:], gb[:, :, 128:256],
                        c10[:, t0:t0 + nt, k:k + 1].to_broadcast(
                            [128, nt, 128]),
                        Alu.mult)
                    u3 = vpool.tile([128, nt, 128], BF16, tag="u3", name="u3")
                    nc.vector.tensor_tensor(
                        u3[:], gb[:, :, 384:512],
                        c11[:, t0:t0 + nt, k:k + 1].to_broadcast(
                            [128, nt, 128]),
                        Alu.mult)
                    vb = vpool.tile([128, nt, 128], BF16, tag="vb", name="vb")
                    nc.vector.tensor_tensor(vb[:], u1[:], u2[:], Alu.add)
                    nc.vector.tensor_tensor(vb[:], vb[:], u3[:], Alu.add)
                    nc.vector.tensor_tensor(vb[:], vb[:], mb[:], Alu.add)

                    trp = None
                    for t in range(nt):
                        tg = t0 + t
                        if tg % 4 == 0:
                            trp = tpool.tile([128, 512], BF16, tag="tr",
                                             name="trp")
                        nc.tensor.transpose(
                            trp[:, (tg % 4) * 128:(tg % 4) * 128 + 128],
                            vb[:, t, :], identb[:])
                        if tg % 4 == 3:
                            vT = vpool.tile([128, 512], BF16, tag="vT",
                                            name="vT")
                            nc.scalar.copy(vT[:], trp[:])
                            bk = tg // 4
                            nc.tensor.matmul(
                                outp[:, bk * 512:(bk + 1) * 512], wT[:, k, :],
                                vT[:], start=(k == 0), stop=(k == K2 - 1),
                            )
            for bk in range(4):
                nc.scalar.activation(
                    out_sb[:, h * PPH + bk * 512: h * PPH + (bk + 1) * 512],
                    outp[:, bk * 512:(bk + 1) * 512],
                    IDENT, bias=bias_sb[:], scale=1.0,
                )
            nc.sync.dma_start(
                AP(out_d.tensor, h * PPH, [[HW, 128], [1, PPH]]),
                out_sb[:, h * PPH:(h + 1) * PPH],
            )


def _make_consts():
    c = np.zeros((128, 707), np.float32)
    c[:, 0:128] = np.eye(128, dtype=np.float32)
    p = np.arange(128)
    c[:, 128] = p
    c[:, 129] = (p >= 64)
    c[:, 130] = p % 64
    s = np.arange(32)[:, None, None]
    kyv = np.arange(3)[None, :, None]
    kxv = np.arange(3)[None, None, :]
    c[:, 131:419] = np.broadcast_to(
        (2 * s + kyv - 1 + 0 * kxv).reshape(-1), (128, 288))
    c[:, 419:707] = np.broadcast_to(
        (0 * s + 0 * kyv + kxv - 1).reshape(-1), (128, 288))
    return c


_COMPILED = None


def _get_compiled():
    global _COMPILED
    if _COMPILED is None:
        nc = bacc.Bacc(get_trn_type() or "TRN2", target_bir_lowering=False,
                       debug=False, num_devices=B, num_swdge_queues=4)
        with tile.TileContext(nc) as tc:
            _emit(tc)
        nc.compile()
        _COMPILED = nc
    return _COMPILED


def kernel(x, w_om, b_om, weight, bias):
    global LAST_EXEC_TIME_NS, LAST_RESULT
    x = np.ascontiguousarray(np.asarray(x, dtype=np.float32))
    w_om_f = np.ascontiguousarray(np.asarray(w_om, np.float32).reshape(27, 1152))
    b_om_f = np.ascontiguousarray(np.asarray(b_om, np.float32).reshape(27, 1))
    weight_f = np.ascontiguousarray(np.asarray(weight, np.float32).reshape(128, 1152))
    bias_f = np.ascontiguousarray(np.asarray(bias, np.float32).reshape(128, 1))

    nc = _get_compiled()
    consts = _make_consts()
    in_maps = [
        {
            "x": np.ascontiguousarray(x[b].reshape(C, HW)),
            "w_om": w_om_f,
            "b_om": b_om_f,
            "weight": weight_f,
            "bias": bias_f,
            "consts": consts,
        }
        for b in range(B)
    ]
    trace = bool(os.environ.get("DCN_TRACE"))
    res = run_bass_kernel_spmd(nc, in_maps, core_ids=list(range(B)), trace=trace)
    LAST_RESULT = res
    LAST_EXEC_TIME_NS = res.exec_time_ns
    out = np.stack([res.results[b]["out"].reshape(C, H, W) for b in range(B)])
    return out.astype(np.float32)


# revision 12
# speedup vs baseline: 1.5044x; 1.5044x over previous
"""DCNv2 (modulated deformable convolution) on 8 Trainium2 NeuronCores.

kernel(**inputs) takes the full unsharded inputs
    x      (8, 128, 64, 64) f32
    w_om   (27, 128, 3, 3)  f32
    b_om   (27,)            f32
    weight (128, 128, 3, 3) f32
    bias   (128,)           f32
and returns the full output (8, 128, 64, 64) f32.

Sharding: pure data-parallel over batch - one image per NeuronCore, small
weights replicated; no collectives.

v9 per-core program (bf16 datapath):
  1. x/weights are cast to bf16 during the load DMA (SWDGE); x is staged
     twice into a DRAM image xt2[GROWS, 256] where row r=(y,x) holds
     [C(y,x), C(y+1,x)] - the 4 bilinear corners of any sample are 4*128
     CONTIGUOUS bf16 values (one 1KB gather descriptor per (pixel, tap)).
     Staging is fully DMA/PE-light: 32 PE transposes land row-pair tiles
     in PSUM, the DVE duplicates them into (B|A) pairs, and 16 big strided
     DMAs write both copies; pad rows are zero-filled separately (disjoint
     regions, no WAW with the staged interior).
  2. offset conv (27ch 3x3) on the PE in bf16, split into two halves of
     the image so the whole index pipeline (sampling positions -> gather
     row indices -> 16-partition dma_gather wrap) completes for half 0
     while half 1 is still in the offset conv; gather descriptor
     generation (the serial Q7 bottleneck) starts ~50us earlier than a
     monolithic setup would allow.
  3. per (half, tap) one dma_gather (SWDGE queues round-robined 0-3)
     fetches [A0 B0 A1 B1] corner blocks in (pixel-partition, channel)
     layout; corners are combined at whole-gather granularity: ACT applies
     c00 per pixel-tile, DVE does 3 broadcast-coefficient multiplies (bf16
     coefficients - pure-bf16 ops keep the DVE 2x datapath) + 3 bf16 adds;
     the result is PE-transposed back to (channel, pixel) and accumulated
     over the 9 taps into PSUM with the 128x128x3x3 weight; bias is added
     on the PSUM->SBUF copy.
"""

import os
import sys

import numpy as np

sys.path.insert(0, "/opt/trn_rl_repo")

from contextlib import ExitStack

import concourse.bacc as bacc
import concourse.mybir as mybir
import concourse.tile as tile
from concourse._compat import get_trn_type
from concourse.alu_op_type import AluOpType as Alu
from concourse.bass import AP
from concourse.bass_utils import run_bass_kernel_spmd
from concourse import library_config

F32 = mybir.dt.float32
BF16 = mybir.dt.bfloat16
I32 = mybir.dt.int32
I16 = mybir.dt.int16

B = 8
C = 128
H = W = 64
HW = H * W
K2 = 9
PADG = 4
GW = H + 2 * PADG      # 72
GROWS = GW * GW        # 5184
NS = 32
NHALF = 2
SPH = NS // NHALF      # 16 s-tiles per half
PPH = HW // NHALF      # 2048 pixels per half
IDENT = mybir.ActivationFunctionType.Identity

LAST_EXEC_TIME_NS = None
LAST_RESULT = None
SINGLE_PACKET = bool(int(os.environ.get("DCN_SP", "0")))


def _emit(tc):
    nc = tc.nc
    x_d = nc.dram_tensor("x", [C, HW], F32, kind="ExternalInput").ap()
    w_om_d = nc.dram_tensor("w_om", [27, 1152], F32, kind="ExternalInput").ap()
    b_om_d = nc.dram_tensor("b_om", [27, 1], F32, kind="ExternalInput").ap()
    weight_d = nc.dram_tensor("weight", [C, 1152], F32, kind="ExternalInput").ap()
    bias_d = nc.dram_tensor("bias", [C, 1], F32, kind="ExternalInput").ap()
    out_d = nc.dram_tensor("out", [C, HW], F32, kind="ExternalOutput").ap()
    xt2_d = nc.dram_tensor("xt2_pad", [GROWS, 256], BF16, kind="Internal").ap()
    consts_d = nc.dram_tensor("consts", [128, 707], F32, kind="ExternalInput").ap()

    ctx = ExitStack()
    with ctx:
        cpool = ctx.enter_context(tc.tile_pool(name="const", bufs=1))
        spool = ctx.enter_context(tc.tile_pool(name="setup", bufs=1))
        stgpool = ctx.enter_context(tc.tile_pool(name="stg", bufs=2))
        dpool = ctx.enter_context(tc.tile_pool(name="data", bufs=1))
        gpool = ctx.enter_context(tc.tile_pool(name="gath", bufs=4))
        vpool = ctx.enter_context(tc.tile_pool(name="val", bufs=2))
        ppool = ctx.enter_context(tc.tile_pool(name="psum", bufs=1, space="PSUM"))
        tpool = ctx.enter_context(tc.tile_pool(name="trps", bufs=2, space="PSUM"))
        opool = ctx.enter_context(tc.tile_pool(name="omps", bufs=2, space="PSUM"))

        # ---------- loads ----------
        cons = cpool.tile([128, 707], F32)
        nc.sync.dma_start(cons[:], consts_d[:, :])
        ident = cons[:, 0:128]
        hob = cons[:, 129:130]
        wo_r = cons[:, 130:131]
        ykc = cons[:, 131:419]
        xkc = cons[:, 419:707]

        x16 = spool.tile([128, HW], BF16)
        nc.gpsimd.dma_start(x16[:], x_d[:, :])
        w_om16 = spool.tile([27, 1152], BF16)
        nc.gpsimd.dma_start(w_om16[:], w_om_d[:, :])
        w16 = spool.tile([128, 1152], BF16)
        nc.gpsimd.dma_start(w16[:], weight_d[:, :])
        nc.gpsimd.load_library(library_config.mlp)

        b_om_sb = spool.tile([27, 1], F32)
        nc.sync.dma_start(b_om_sb[:], b_om_d[:, :])
        bias_sb = spool.tile([128, 1], F32)
        nc.sync.dma_start(bias_sb[:], bias_d[:, :])

        identb = spool.tile([128, 128], BF16)
        nc.vector.tensor_copy(identb[:], ident)

        # ---------- zero-fill xt2 pad regions (disjoint from staged rows) --
        zt = spool.tile([128, 576], BF16)
        nc.vector.memset(zt[:], 0.0)
        # top rows 0..3 (incl. row 3 cols 0:128 = C(-1); cols 128:256 of row
        # 3 are overwritten by the B-copy of chunk 0 afterwards)
        nc.scalar.dma_start(AP(xt2_d.tensor, 0, [[576, 128], [1, 576]]), zt[:])
        # bottom rows 68..71
        nc.scalar.dma_start(
            AP(xt2_d.tensor, 68 * GW * 256, [[576, 128], [1, 576]]), zt[:])
        # left pad cols x'=0..3 of interior rows
        nc.scalar.dma_start(
            AP(xt2_d.tensor, 4 * GW * 256, [[GW * 256, 64], [1, 1024]]),
            zt[:, 0:512])
        # right pad cols x'=68..71 of interior rows
        nc.scalar.dma_start(
            AP(xt2_d.tensor, (4 * GW + 68) * 256, [[GW * 256, 64], [1, 1024]]),
            zt[:, 0:512])
        # row 67 cols 128:256 = C(64) = 0 (not covered by B-copies)
        nc.scalar.dma_start(
            AP(xt2_d.tensor, (67 * GW + 4) * 256 + 128, [[256, 64], [1, 128]]),
            zt[0:64, 0:128])

        # ---------- stage xt2 interior ----------
        # xt2 row r = (y+PADG, x+PADG) holds [C(y,x), C(y+1,x)].  Chunk
        # s covers image rows (2s, 2s+1): PE-transpose to [pixel, chan],
        # copy to SBUF, then per (4-chunk group, y2-parity) two strided
        # DMAs write the same slab twice:
        #   A: row 2s+4+y2 cols   0:128  = C(2s+y2)
        #   B: row 2s+3+y2 cols 128:256  = C(2s+y2)
        for g in range(8):
            trp4 = tpool.tile([128, 512], BF16, tag="tr", name="trp")
            for s4 in range(4):
                s = 4 * g + s4
                nc.tensor.transpose(
                    trp4[:, 128 * s4:128 * s4 + 128],
                    x16[:, 128 * s:128 * s + 128], identb[:])
            stg4 = stgpool.tile([128, 512], BF16, tag="stg", name="stg")
            nc.vector.tensor_copy(stg4[:], trp4[:])
            stg4v = stg4[:].rearrange("p (a b) -> p a b", a=4)
            for y2 in range(2):
                eng = nc.sync if y2 == 0 else nc.scalar
                src = stg4v[64 * y2:64 * y2 + 64, :, :]
                eng.dma_start(
                    AP(xt2_d.tensor, ((8 * g + 4 + y2) * GW + 4) * 256,
                       [[256, 64], [2 * GW * 256, 4], [1, 128]]),
                    src,
                )
                eng.dma_start(
                    AP(xt2_d.tensor, ((8 * g + 3 + y2) * GW + 4) * 256 + 128,
                       [[256, 64], [2 * GW * 256, 4], [1, 128]]),
                    src,
                )

        # ---------- x_pad (bf16) for the offset conv ----------
        XP = 66
        x_pad = spool.tile([128, XP * XP], BF16)
        xpv = x_pad[:].rearrange("p (a b) -> p a b", a=XP)
        nc.vector.memset(xpv[:, 0:1, :], 0.0)
        nc.vector.memset(xpv[:, 65:66, :], 0.0)
        nc.vector.memset(xpv[:, 1:65, 0:1], 0.0)
        nc.vector.memset(xpv[:, 1:65, 65:66], 0.0)
        nc.vector.tensor_copy(
            xpv[:, 1:65, 1:65],
            x16[:].rearrange("p (a b) -> p a b", a=64),
        )

        # ---------- weight transposes (offset conv) ----------
        womT = spool.tile([128, K2, 27], BF16)
        for k in range(K2):
            trp = tpool.tile([128, 512], BF16, tag="tr", name="trp")
            nc.tensor.transpose(
                trp[:, 0:27],
                w_om16[:].rearrange("p (c k) -> p c k", k=K2)[:, :, k],
                identb[0:27, 0:27],
            )
            nc.scalar.copy(womT[:, k, :], trp[:, 0:27])

        om_sb = spool.tile([27, HW], BF16)
        omT = spool.tile([128, NS, 27], F32)
        idxAw = spool.tile([128, K2 * 256], I16)
        omT_t = omT[:].tensor
        omT_off = omT[:].offset

        _cnt = [0]

        def f(shape=(128, SPH, K2), dt=F32, tag=None):
            _cnt[0] += 1
            nm = f"cf{_cnt[0]}"
            return dpool.tile(list(shape), dt, tag=tag or nm, name=nm)

        def om_conv_half(h):
            # offset conv chunks + per-s4 omT transposes for one half
            for ch in range(4 * h, 4 * h + 4):
                omp = opool.tile([128, 512], F32, tag="om", name="omp")
                for k in range(K2):
                    dy_, dx_ = k // 3, k % 3
                    r0 = ch * 8 + dy_
                    nc.tensor.matmul(
                        omp[0:27, :], womT[:, k, :],
                        xpv[:, r0:r0 + 8, dx_:dx_ + 64],
                        start=(k == 0), stop=(k == K2 - 1),
                    )
                nc.scalar.activation(
                    om_sb[:, ch * 512:(ch + 1) * 512], omp[0:27, :],
                    IDENT, bias=b_om_sb[:], scale=1.0,
                )
                # omT (128 pix, 27) for the 4 s-tiles of this chunk
                trp = tpool.tile([128, 512], BF16, tag="tr", name="trp")
                for j in range(4):
                    nc.tensor.transpose(
                        trp[:, j * 128:j * 128 + 27],
                        om_sb[:, (4 * ch + j) * 128:(4 * ch + j + 1) * 128],
                        identb[0:27, 0:27],
                    )
                nc.scalar.copy(
                    omT[:, 4 * ch:4 * ch + 4, :],
                    trp[:].rearrange("p (a b) -> p a b", b=128)[:, :, 0:27],
                )

        def index_half(h):
            # sampling positions -> gather row indices for s-tiles
            # h*16..h*16+15; returns (py, px, y0f, x0f, mlg)
            so = h * SPH
            dyT = AP(omT_t, omT_off + so * 27,
                     [[NS * 27, 128], [27, SPH], [2, K2]])
            dxT = AP(omT_t, omT_off + so * 27 + 1,
                     [[NS * 27, 128], [27, SPH], [2, K2]])
            ykv = ykc.rearrange("p (s a) -> p s a", a=K2)[:, so:so + SPH, :]
            xkv = xkc.rearrange("p (s a) -> p s a", a=K2)[:, so:so + SPH, :]
            py = f(tag=f"py{h}")
            nc.vector.scalar_tensor_tensor(py[:], dyT, hob, ykv, Alu.add,
                                           Alu.add)
            px = f(tag=f"px{h}")
            nc.vector.scalar_tensor_tensor(px[:], dxT, wo_r, xkv, Alu.add,
                                           Alu.add)

            def floorit(v, nm):
                vi = f(dt=I32, tag=f"fl_i{h}")
                nc.vector.tensor_copy(vi[:], v[:])
                v0 = f(tag=f"fl_f{h}")
                nc.vector.tensor_copy(v0[:], vi[:])
                gt = f(tag=f"fl_gt{h}")
                nc.vector.tensor_tensor(gt[:], v0[:], v[:], Alu.is_gt)
                v0f = f(tag=nm)
                nc.vector.tensor_tensor(v0f[:], v0[:], gt[:], Alu.subtract)
                return v0f

            y0f = floorit(py, f"y0f{h}")
            x0f = floorit(px, f"x0f{h}")
            nc.vector.tensor_scalar(y0f[:], y0f[:], -float(PADG), float(H + 2),
                                    Alu.max, Alu.min)
            nc.vector.tensor_scalar(x0f[:], x0f[:], -float(PADG), float(W + 2),
                                    Alu.max, Alu.min)

            # row index r = (y0+PADG)*GW + (x0+PADG), k-major: gKM[p][k][s]
            gAf = f(tag=f"gA{h}")
            nc.vector.tensor_scalar(gAf[:], y0f[:], float(GW),
                                    float(PADG * GW + PADG),
                                    Alu.mult, Alu.add)
            gKM = dpool.tile([128, K2, SPH], F32, tag=f"gKM{h}",
                             name=f"gKM{h}")
            gKM_w = AP(gKM[:].tensor, gKM[:].offset,
                       [[K2 * SPH, 128], [1, SPH], [SPH, K2]])
            nc.vector.tensor_tensor(gKM_w, gAf[:], x0f[:], Alu.add)
            return py, px, y0f, x0f, gKM

        def wrap_half(h, gKM):
            # idx wrap via PE transposes: idxAw[16u+pp][k*256 + h*128 +
            # s*8 + u] = gKM[16u+pp][k][s]
            t1s = []
            gv = gKM[:].rearrange("p a b -> p (a b)")
            for g in range(3):  # pass 1: [128, 48] -> [48, 128]
                trp = opool.tile([128, 512], F32, tag="om", name="omp")
                nc.tensor.transpose(
                    trp[0:48, 0:128], gv[:, 48 * g:48 * (g + 1)], ident[:])
                t1 = spool.tile([48, 128], F32, tag=f"t1_{h}_{g}")
                nc.scalar.copy(t1[:], trp[0:48, 0:128])
                t1s.append(t1)
            for g in range(3):
                for u4 in range(2):  # pass 2: 4x [48, 16] -> [16, 48]
                    trp = opool.tile([128, 512], F32, tag="om", name="omp")
                    for j in range(4):
                        u = 4 * u4 + j
                        nc.tensor.transpose(
                            trp[0:16, j * 128:j * 128 + 48],
                            t1s[g][:, 16 * u:16 * u + 16],
                            ident[0:48, 0:48],
                        )
                    t2 = vpool.tile([16, 512], F32, tag="t2", name="t2")
                    nc.scalar.copy(t2[:], trp[0:16, :])
                    # scatter (j, k', s) -> col (3g+k')*256 + h*128 + s*8 + u
                    dst = AP(
                        idxAw[:].tensor,
                        idxAw[:].offset + (3 * g) * 256 + 128 * h + 4 * u4,
                        [[K2 * 256, 16], [1, 4], [256, 3], [8, SPH]],
                    )
                    src = AP(
                        t2[:].tensor, t2[:].offset,
                        [[512, 16], [128, 4], [16, 3], [1, 16]],
                    )
                    nc.vector.tensor_copy(dst, src)
            # replicate this half's index columns to all 8 16-row groups
            iv = idxAw[:].rearrange("p (k g c) -> p k g c", k=K2, g=2)
            for u2 in range(1, 8):
                nc.scalar.dma_start(
                    iv[16 * u2:16 * u2 + 16, :, h, :], iv[0:16, :, h, :])

        def coef_half(h, py, px, y0f, x0f):
            # softmax mask + bilinear coefficients (c00 f32 for the ACT
            # scale; the rest bf16 so the DVE combine stays pure-bf16)
            mlg = omT[:, h * SPH:(h + 1) * SPH, 18:27]
            e = f(tag=f"e{h}")
            nc.scalar.activation(e[:], mlg, mybir.ActivationFunctionType.Exp)
            ssum = f((128, SPH, 1), tag=f"ss{h}")
            nc.vector.tensor_reduce(ssum[:], e[:], mybir.AxisListType.X,
                                    Alu.add)
            rs = f((128, SPH, 1), tag=f"rs{h}")
            nc.vector.reciprocal(rs[:], ssum[:])
            mask = f(tag=f"mask{h}")
            nc.vector.tensor_tensor(mask[:], e[:],
                                    rs[:].to_broadcast([128, SPH, K2]),
                                    Alu.mult)

            wy1 = f(tag=f"wy1{h}")
            nc.vector.tensor_tensor(wy1[:], py[:], y0f[:], Alu.subtract)
            wy0 = f(tag=f"wy0{h}")
            nc.vector.tensor_scalar(wy0[:], wy1[:], -1.0, 1.0, Alu.mult,
                                    Alu.add)
            wx1 = f(tag=f"wx1{h}")
            nc.vector.tensor_tensor(wx1[:], px[:], x0f[:], Alu.subtract)
            wx0 = f(tag=f"wx0{h}")
            nc.vector.tensor_scalar(wx0[:], wx1[:], -1.0, 1.0, Alu.mult,
                                    Alu.add)

            mwy0 = f(tag=f"mwy0{h}")
            nc.vector.tensor_tensor(mwy0[:], mask[:], wy0[:], Alu.mult)
            mwy1 = f(tag=f"mwy1{h}")
            nc.vector.tensor_tensor(mwy1[:], mask[:], wy1[:], Alu.mult)
            c00 = f(tag=f"c00{h}")
            nc.vector.tensor_tensor(c00[:], mwy0[:], wx0[:], Alu.mult)

            # c01/c10/c11 in bf16 with each value DUPLICATED into a pair:
            # the combine multiplies then broadcast them with a real
            # innermost stride-1 pair so the DVE 16-bit 2x datapath stays
            # engaged (a plain stride-0 broadcast operand forces 1x mode).
            def dup2(t):
                a = t[:]
                return AP(a.tensor, a.offset,
                          [[SPH * K2, 128], [K2, SPH], [1, K2], [0, 2]])

            def cdup(wyt, wxt, nm):
                cd = f((128, SPH, K2, 2), dt=BF16, tag=nm)
                nc.vector.tensor_tensor(cd[:], dup2(wyt), dup2(wxt), Alu.mult)
                return cd

            c01 = cdup(mwy0, wx1, f"c01{h}")
            c10 = cdup(mwy1, wx0, f"c10{h}")
            c11 = cdup(mwy1, wx1, f"c11{h}")
            return c00, c01, c10, c11

        # ---------- setup, half 0 first so gathers start early ----------
        om_conv_half(0)
        py0, px0, y0f0, x0f0, gKM0 = index_half(0)
        wrap_half(0, gKM0)
        cs0 = coef_half(0, py0, px0, y0f0, x0f0)

        om_conv_half(1)

        wT = spool.tile([128, K2, 128], BF16)
        for k in range(K2):
            trp = tpool.tile([128, 512], BF16, tag="tr", name="trp")
            nc.tensor.transpose(
                trp[:, 0:128],
                w16[:].rearrange("p (c k) -> p c k", k=K2)[:, :, k], identb[:],
            )
            nc.scalar.copy(wT[:, k, :], trp[:, 0:128])

        py1, px1, y0f1, x0f1, gKM1 = index_half(1)
        wrap_half(1, gKM1)
        cs1 = coef_half(1, py1, px1, y0f1, x0f1)
        coefs = [cs0, cs1]

        # Gate the gather storm on the full DVE index/coef pipeline: small
        # int-cast/clamp DVE ops that run concurrently with DMAGatherAnt
        # descriptor generation block until the generation finishes (SBUF
        # ring arbitration), stalling the DVE and starving the combine.
        # Rewriting idxAw with +0 (derived from the last coefficient tile)
        # gives every dma_gather a data dependency on the completed setup.
        zi = dpool.tile([128, 1], I16, tag="zi", name="zi")
        nc.vector.tensor_tensor(zi[:], cs1[3][:, 0, 0, 0:1],
                                cs1[3][:, 0, 0, 0:1], Alu.subtract)
        nc.vector.tensor_tensor(idxAw[:], idxAw[:],
                                zi[:].to_broadcast([128, K2 * 256]), Alu.add)

        # ---------- main loop ----------
        out_sb = spool.tile([128, HW], F32)
        xt2_src = AP(xt2_d.tensor, 0, [[256, GROWS - 1], [1, 512]])
        for h in range(NHALF):
            c00, c01, c10, c11 = coefs[h]
            outp = ppool.tile([128, PPH], F32, tag="out", name="outp")
            for k in range(K2):
                # split the last tap (tail) and the very first gather
                # (startup ramp) into two half-gathers on separate queues
                split = (k == K2 - 1) or (h == 0 and k == 0)
                parts = ((0, 8), (8, SPH)) if split else ((0, SPH),)
                for (t0, t1) in parts:
                    nt = t1 - t0
                    gb = gpool.tile([128, nt, 512], BF16, tag="gb", name="gb")
                    nc.gpsimd.dma_gather(
                        gb[:], xt2_src,
                        idxAw[:, k * 256 + 128 * h + 8 * t0:
                              k * 256 + 128 * h + 8 * t0 + 8 * nt],
                        128 * nt, 128 * nt, 512, elem_step=256,
                        single_packet=SINGLE_PACKET,
                        queue_num=(h * K2 + k + t0 // 8) % 4,
                    )
                    # corners: [0:128]=A0(c00) [128:256]=B0(c10)
                    #          [256:384]=A1(c01) [384:512]=B1(c11)
                    mb = vpool.tile([128, nt, 128], BF16, tag="mb", name="mb")
                    for t in range(nt):
                        s = t0 + t
                        nc.scalar.activation(mb[:, t, :], gb[:, t, 0:128],
                                             IDENT, bias=0.0,
                                             scale=c00[:, s, k:k + 1])

                    def pairs(a, off):
                        # [128, nt, 128] slab viewed as [128, nt, 64, 2]
                        return AP(a.tensor, a.offset + off,
                                  [[a.ap[0][0], 128], [512, nt], [2, 64],
                                   [1, 2]])

                    def cpairs(cdt):
                        # [128, nt, 64, 2]: per-(s,k) coefficient pair
                        # broadcast over the 64 channel-pairs
                        a = cdt[:]
                        return AP(a.tensor,
                                  a.offset + (t0 * K2 + k) * 2,
                                  [[SPH * K2 * 2, 128], [K2 * 2, nt],
                                   [0, 64], [1, 2]])

                    gba = gb[:]

                    def upair(tag, corner_off, cdt):
                        u = vpool.tile([128, nt, 128], BF16, tag=tag,
                                       name=tag)
                        ua = u[:]
                        nc.vector.tensor_tensor(
                            AP(ua.tensor, ua.offset,
                               [[nt * 128, 128], [128, nt], [2, 64], [1, 2]]),
                            pairs(gba, corner_off), cpairs(cdt), Alu.mult)
                        return u

                    u1 = upair("u1", 256, c01)
                    u2 = upair("u2", 128, c10)
                    u3 = upair("u3", 384, c11)
                    vb = vpool.tile([128, nt, 128], BF16, tag="vb", name="vb")
                    nc.vector.tensor_tensor(vb[:], u1[:], u2[:], Alu.add)
                    nc.vector.tensor_tensor(vb[:], vb[:], u3[:], Alu.add)
                    nc.vector.tensor_tensor(vb[:], vb[:], mb[:], Alu.add)

                    trp = None
                    for t in range(nt):
                        tg = t0 + t
                        if tg % 4 == 0:
                            trp = tpool.tile([128, 512], BF16, tag="tr",
                                             name="trp")
                        nc.tensor.transpose(
                            trp[:, (tg % 4) * 128:(tg % 4) * 128 + 128],
                            vb[:, t, :], identb[:])
                        if tg % 4 == 3:
                            vT = vpool.tile([128, 512], BF16, tag="vT",
                                            name="vT")
                            nc.scalar.copy(vT[:], trp[:])
                            bk = tg // 4
                            nc.tensor.matmul(
                                outp[:, bk * 512:(bk + 1) * 512], wT[:, k, :],
                                vT[:], start=(k == 0), stop=(k == K2 - 1),
                            )
            for bk in range(4):
                nc.scalar.activation(
                    out_sb[:, h * PPH + bk * 512: h * PPH + (bk + 1) * 512],
                    outp[:, bk * 512:(bk + 1) * 512],
                    IDENT, bias=bias_sb[:], scale=1.0,
                )
            nc.sync.dma_start(
                AP(out_d.tensor, h * PPH, [[HW, 128], [1, PPH]]),
                out_sb[:, h * PPH:(h + 1) * PPH],
            )


def _make_consts():
    c = np.zeros((128, 707), np.float32)
    c[:, 0:128] = np.eye(128, dtype=np.float32)
    p = np.arange(128)
    c[:, 128] = p
    c[:, 129] = (p >= 64)
    c[:, 130] = p % 64
    s = np.arange(32)[:, None, None]
    kyv = np.arange(3)[None, :, None]
    kxv = np.arange(3)[None, None, :]
    c[:, 131:419] = np.broadcast_to(
        (2 * s + kyv - 1 + 0 * kxv).reshape(-1), (128, 288))
    c[:, 419:707] = np.broadcast_to(
        (0 * s + 0 * kyv + kxv - 1).reshape(-1), (128, 288))
    return c


_COMPILED = None


def _get_compiled():
    global _COMPILED
    if _COMPILED is None:
        nc = bacc.Bacc(get_trn_type() or "TRN2", target_bir_lowering=False,
                       debug=False, num_devices=B, num_swdge_queues=4)
        with tile.TileContext(nc) as tc:
            _emit(tc)
        nc.compile()
        _COMPILED = nc
    return _COMPILED


def kernel(x, w_om, b_om, weight, bias):
    global LAST_EXEC_TIME_NS, LAST_RESULT
    x = np.ascontiguousarray(np.asarray(x, dtype=np.float32))
    w_om_f = np.ascontiguousarray(np.asarray(w_om, np.float32).reshape(27, 1152))
    b_om_f = np.ascontiguousarray(np.asarray(b_om, np.float32).reshape(27, 1))
    weight_f = np.ascontiguousarray(np.asarray(weight, np.float32).reshape(128, 1152))
    bias_f = np.ascontiguousarray(np.asarray(bias, np.float32).reshape(128, 1))

    nc = _get_compiled()
    consts = _make_consts()
    in_maps = [
        {
            "x": np.ascontiguousarray(x[b].reshape(C, HW)),
            "w_om": w_om_f,
            "b_om": b_om_f,
            "weight": weight_f,
            "bias": bias_f,
            "consts": consts,
        }
        for b in range(B)
    ]
    trace = bool(os.environ.get("DCN_TRACE"))
    res = run_bass_kernel_spmd(nc, in_maps, core_ids=list(range(B)), trace=trace)
    LAST_RESULT = res
    LAST_EXEC_TIME_NS = res.exec_time_ns
    out = np.stack([res.results[b]["out"].reshape(C, H, W) for b in range(B)])
    return out.astype(np.float32)
